# revision 2
# baseline (speedup 1.0000x reference)
"""DFRNN forward kernel v3 for TRN2 (8 NeuronCores, data-parallel over num_ts).

Per token z = X[ts, t, :] (128 feats):
  global: 3-layer LSTM single-step-from-zero (hidden 512), f-gate dead
  mu    = relu(h3) . wsum + b_sum
  noise: 2-layer stack (hidden 128); sigma = softplus(st) + 1e-6

v3 changes vs v2:
  - L1/L2 i,o gate matmuls in fp8e4 DoubleRow (2 contraction rows/cycle);
    c-gate stays f16 (tanh' = 1 makes it the error-sensitive path).
  - h tiles produced twice: f16 (c-gate rhs / mu) on DVE, fp8 (i,o rhs)
    on GPSIMD.
  - custom DVE uops: ANT_VF/ANT_VFR fuse cc = si*tc (+relu) and the
    deg-5 tanh(cc) poly into ONE DVE pass; ANT_SIG5C evaluates a deg-5
    centered sigmoid (sig(x)-0.5) with per-partition bias straight from
    PSUM, offloading part of the ACT sigmoid load onto DVE.
  - h = (so_c + 0.5) * v via scalar_tensor_tensor where sigma(o) was
    offloaded (the +0.5 rides the STT scalar slot).
Engine balance (per superblock): ACT ~33 activations, DVE ~14 VF + 9
SIG5C + 10 h-mults, GPSIMD 8 fp8 h-mults + 4 f16 h-mults, PE fp8+f16 mix.
"""

import contextlib

import numpy as np
import ml_dtypes

import concourse.bass as bass
import concourse.mybir as mybir
import concourse.tile as tile
from concourse import bacc, bass_utils
from concourse import dve_ops
from concourse.dve_spec import (Spec, Src0, Src1, C0, C1, C2, C3, One, lower,
                                relu as dve_relu, _spill_c3_to_src1, _has_src1)
from concourse.dve_uop import DveOpSpec

P = 128
IN = 128
GH = 512
NH = 128
NCORES = 8
FULL_NT = 1024
T = 192

SB = 1024              # tokens per superblock
CHUNKS = SB // P       # 8
NBK = SB // 512        # psum 512-col chunks (f16 path)
NBK8 = SB // 256       # psum 256-col chunks (fp8 DoubleRow path)

F32 = mybir.dt.float32
F16 = mybir.dt.float16
F8 = mybir.dt.float8e4
NF8 = ml_dtypes.float8_e4m3
AF = mybir.ActivationFunctionType
ALU = mybir.AluOpType
DR = mybir.MatmulPerfMode.DoubleRow

# tanh(x)/x on [-1,1]: deg-2 in u=x^2 (max tanh err 3.9e-4)
PC = (0.99716005, -0.30797734, 0.07279899)
# (sig(x)-0.5)/x deg-2 in u=x^2, fit on [-4.5,4.5] (max err 8.3e-3)
QC = (0.23441365, -0.01149145, 0.00026533)

GATES = ("i", "c", "o")

# tiles: ("g", l, j) for l in 0..2, j in 0..3; ("n", l, 0) for l in 0..1
# sigma(o) offloaded to DVE SIG5D for these tiles (SIG5D reads PSUM at
# half DVE rate, ~3.1us vs ACT's 1.11us -- only worth ~5 offloads):
OFF_SO = set()
# f16 h-mult on GPSIMD for these tiles (rest on DVE). L0's f16 h is
# consumed a full pipeline step later (L1 c-gate), so GPSIMD latency is
# hidden; gt (L2) feeds mu at the end of the same step and must stay on
# the fast DVE path.
H16_GPS = {("g", 0, 0), ("g", 0, 1), ("g", 0, 2), ("g", 0, 3)}


# ---------------- custom DVE ops (runtime registration) ----------------

def _register(name, spec, subdim=False):
    by = {op.name: op for op in dve_ops.OPS}
    if name in by:
        return by[name]
    row = dve_ops._CUSTOM_DVE_ROW_BASE + len(dve_ops.OPS)
    assert row < 0x20, "custom-DVE opcode rows exhausted"
    shas = {}
    for ver in ("v3", "v4"):
        s = DveOpSpec(name=name, opcode=row, uops=lower(spec, ver=ver),
                      rd1_en=_has_src1(spec))
        shas[ver] = s.sha(ver)
    op = dve_ops.DveOp(name, spec, subdim, uops_sha=shas)
    dve_ops.OPS.append(op)
    dve_ops.CUSTOM_DVE_SPECS[name] = spec
    dve_ops._SUB_OPCODE_FOR_NAME[name] = row
    return op


def _vf_ref(in0, in1, s0, s1, imm2):
    cc = (in0 * in1).astype(np.float32)
    u = cc * cc
    return ((u * imm2 + s1) * u + s0) * cc


def _vfr_ref(in0, in1, s0, s1, imm2):
    cc = np.maximum((in0 * in1).astype(np.float32), 0)
    u = cc * cc
    return ((u * imm2 + s1) * u + s0) * cc


def _sig5d_ref(in0, in1, s0, s1, imm2):
    y = (in0 + s0).astype(np.float32)
    u = y * y
    return ((u * in1 + imm2) * u + s1) * y + 1.0


def _make_ops():
    cc = Src0 * Src1
    u = cc * cc
    vf = _register("ANT_VF", Spec(body=((u * C2 + C1) * u + C0) * cc,
                                  reference=_vf_ref))
    ccr = dve_relu(Src0 * Src1)
    ur = ccr * ccr
    vfr = _register("ANT_VFR", Spec(body=((ur * C2 + C1) * ur + C0) * ccr,
                                    reference=_vfr_ref))
    # 2*sigmoid(in0 + bias): doubled q-coefs at the call site; +1 via One
    y = Src0 + C0
    uy = y * y
    sig5d = _register("ANT_SIG5D", Spec(
        body=_spill_c3_to_src1(((uy * C3 + C2) * uy + C1) * y + One),
        reference=_sig5d_ref))
    return vf, vfr, sig5d


OP_VF, OP_VFR, OP_SIG5D = _make_ops()


# ---------------- slot tables ----------------

def _slot16_table():
    """f16 weight slots: (kind, l, g, j, k) -> slot idx (each 128 wide)."""
    slots = []
    for g in GATES:                       # global L0
        for j in range(4):
            slots.append(("g", 0, g, j, 0))
    for l in (1, 2):                      # global L1/L2 c-gate
        for j in range(4):
            for k in range(4):
                slots.append(("g", l, "c", j, k))
    for l in (0, 1):                      # noise
        for g in GATES:
            slots.append(("n", l, g, 0, 0))
    return {s: i for i, s in enumerate(slots)}, len(slots)


S16, N16 = _slot16_table()
VEC_SLOT = N16                 # wsum cols 0-3, a_w col 4
W16W = (N16 + 1) * P


def _slot8(l, g, j, kp):
    """fp8 slot: (l in {1,2}, g in {i,o}, j, kp) -> pair-slot idx."""
    gi = 0 if g == "i" else 1
    return (((l - 1) * 2 + gi) * 4 + j) * 2 + kp


N8 = 32                        # pair slots, each [P, 2, 128]
NBIAS = 48


def _bias_col(kind, l, g, j):
    gi = GATES.index(g)
    if kind == "g":
        return (l * 3 + gi) * 4 + j
    return 36 + l * 3 + gi


# ---------------- host packing ----------------

def pack_host(g_Wih0, g_bih0, g_bhh0, g_Wih, g_bih, g_bhh, e_W, e_b,
              n_Wih0, n_bih0, n_bhh0, n_Wih, n_bih, n_bhh, a_W, a_b):
    g_off = {"i": 0, "c": 2 * GH, "o": 3 * GH}
    n_off = {"i": 0, "c": 2 * NH, "o": 3 * NH}

    wt16 = np.zeros((P, W16W), np.float16)
    for (kind, l, g, j, k), si in S16.items():
        if kind == "g":
            W = g_Wih0 if l == 0 else g_Wih[l - 1]
            rows = slice(g_off[g] + j * P, g_off[g] + (j + 1) * P)
        else:
            W = n_Wih0 if l == 0 else n_Wih[0]
            rows = slice(n_off[g] + j * P, n_off[g] + (j + 1) * P)
        wt16[:, si * P:(si + 1) * P] = np.asarray(W)[rows, k * P:(k + 1) * P].T
    wsum = np.asarray(e_W).sum(axis=0)
    base = VEC_SLOT * P
    for j in range(4):
        wt16[:, base + j] = wsum[j * P:(j + 1) * P]
    wt16[:, base + 4] = np.asarray(a_W)[0]

    # fp8 pair slots: wt8[p, si8*2 + i, m] = W[rows_j + m, (2kp+i)*128 + p]
    wt8 = np.zeros((P, N8 * 2, P), NF8)
    for l in (1, 2):
        W = np.asarray(g_Wih[l - 1], np.float32)
        for g in ("i", "o"):
            for j in range(4):
                rows = slice(g_off[g] + j * P, g_off[g] + (j + 1) * P)
                for kp in range(2):
                    si8 = _slot8(l, g, j, kp)
                    for i in range(2):
                        blk = W[rows, (2 * kp + i) * P:(2 * kp + i + 1) * P]
                        wt8[:, si8 * 2 + i, :] = blk.T.astype(NF8)
    wt8 = wt8.reshape(P, N8 * 2 * P)

    bias = np.zeros((P, NBIAS), np.float32)
    bg0 = np.asarray(g_bih0) + np.asarray(g_bhh0)
    bn0 = np.asarray(n_bih0) + np.asarray(n_bhh0)
    seen = {(k_, l_, g_, j_) for (k_, l_, g_, j_, _) in S16}
    seen |= {("g", l, g, j) for l in (1, 2) for g in ("i", "o")
             for j in range(4)}
    for (kind, l, g, j) in seen:
        if kind == "g":
            b = bg0 if l == 0 else np.asarray(g_bih[l - 1]) + np.asarray(g_bhh[l - 1])
            off = g_off[g]
        else:
            b = bn0 if l == 0 else np.asarray(n_bih[0]) + np.asarray(n_bhh[0])
            off = n_off[g]
        bias[:, _bias_col(kind, l, g, j)] = b[off + j * P: off + (j + 1) * P]

    b_sum = float(np.asarray(e_b).sum())
    a_bias = float(np.asarray(a_b)[0])
    return wt16, wt8, bias, b_sum, a_bias


def pack_x(X):
    """[nt, T, IN] f32 -> transposed f16 [IN, nt*T] (lhs-ready)."""
    Xf = np.asarray(X, np.float32).reshape(-1, IN)
    return np.ascontiguousarray(Xf.T.astype(np.float16))


# ---------------- device kernel ----------------

def build_nc(tok, b_sum, a_bias, repeat=1):
    nsb = tok // SB
    nc = bacc.Bacc("TRN2", target_bir_lowering=False, debug=False)
    x = nc.dram_tensor("x", [P, tok], F16, kind="ExternalInput").ap()
    wt16_d = nc.dram_tensor("wt16", [P, W16W], F16, kind="ExternalInput").ap()
    wt8_d = nc.dram_tensor("wt8", [P, N8 * 2 * P], F8, kind="ExternalInput").ap()
    bias_d = nc.dram_tensor("bias", [P, NBIAS], F32, kind="ExternalInput").ap()
    mu_d = nc.dram_tensor("mu", [P, nsb * CHUNKS], F32, kind="ExternalOutput").ap()
    sg_d = nc.dram_tensor("sigma", [P, nsb * CHUNKS], F32, kind="ExternalOutput").ap()

    pc0, pc1, pc2 = PC
    q0, q1, q2 = QC

    with tile.TileContext(nc) as tc:
        with (
            tc.tile_pool(name="const", bufs=1) as cpool,
            tc.tile_pool(name="xin", bufs=3) as xpool,
            tc.tile_pool(name="h", bufs=2) as hpool,
            tc.tile_pool(name="tmp", bufs=16) as tpool,
            tc.tile_pool(name="ps", bufs=2, space="PSUM") as pspool,
        ):
            wtr = cpool.tile([P, W16W], F16)
            nc.sync.dma_start(wtr[:], wt16_d)
            w8r = cpool.tile([P, N8 * 2, P], F8)
            nc.sync.dma_start(w8r[:], wt8_d)
            biast = cpool.tile([P, NBIAS], F32)
            nc.sync.dma_start(biast[:], bias_d)
            q2t = cpool.tile([P, 1], F32)
            nc.vector.memset(q2t[:], 2 * q2)

            musb = cpool.tile([P, nsb * CHUNKS], F32)
            stsb = cpool.tile([P, nsb * CHUNKS], F32)
            sgsb = cpool.tile([P, nsb * CHUNKS], F32)

            def w16(kind, l, g, j, k):
                si = S16[(kind, l, g, j, k)]
                return wtr[:, si * P:(si + 1) * P]

            def w8(l, g, j, kp):
                si8 = _slot8(l, g, j, kp)
                return w8r[:, si8 * 2:si8 * 2 + 2, :]

            def bcol(kind, l, g, j):
                c = _bias_col(kind, l, g, j)
                return biast[:, c:c + 1]

            wsum_cols = [wtr[:, VEC_SLOT * P + j: VEC_SLOT * P + j + 1]
                         for j in range(4)]
            aw_col = wtr[:, VEC_SLOT * P + 4: VEC_SLOT * P + 5]

            rep_cm = (tc.For_i(0, repeat, 1) if repeat > 1
                      else contextlib.nullcontext())
            with rep_cm:
              zts = {}
              h16 = {}     # (b, l) -> f16 tile [P, 4, SB] (global) / [P, SB] (noise)
              h8s = {}     # (b, l) -> fp8 tile [P, 4, SB] for l in {0, 1}
              hns = {}

              def fetch_zt(b):
                zt = xpool.tile([P, SB], F16, tag="zt")
                nc.sync.dma_start(zt[:], x[:, b * SB:(b + 1) * SB])
                zts[b] = zt

              def emit_tail(tid, kind, l, g_bias_of, ps_i, ps_c, ps_o,
                            hf16_dst, hf8_dst, last):
                    """si/tc (+so) -> v -> h (f16 and optionally fp8).
                    Offloaded-sigma tiles: so = 2*sig(o) on DVE, v = tanh/2
                    (halved VF coefs) so h = so*v stays a plain TT."""
                    off = tid in OFF_SO
                    hv = 0.5 if off else 1.0
                    si = tpool.tile([P, SB], F16, tag="tmp")
                    nc.scalar.activation(si[:], ps_i[:], AF.Sigmoid,
                                         bias=g_bias_of("i"))
                    tcv = tpool.tile([P, SB], F16, tag="tmp")
                    nc.scalar.activation(tcv[:], ps_c[:], AF.Tanh,
                                         bias=g_bias_of("c"))
                    v = tpool.tile([P, SB], F16, tag="tmp")
                    nc.vector._custom_dve(OP_VFR if last else OP_VF,
                                          out=v[:], in0=si[:], in1=tcv[:],
                                          s0=pc0 * hv, s1=pc1 * hv,
                                          imm2=pc2 * hv)
                    so = tpool.tile([P, SB], F16, tag="tmp")
                    if off:
                        nc.vector._custom_dve(OP_SIG5D, out=so[:],
                                              in0=ps_o[:], in1=q2t[:, 0:1],
                                              s0=g_bias_of("o"), s1=2 * q0,
                                              imm2=2 * q1)
                    else:
                        nc.scalar.activation(so[:], ps_o[:], AF.Sigmoid,
                                             bias=g_bias_of("o"))
                    heng = nc.gpsimd if tid in H16_GPS else nc.vector
                    heng.tensor_tensor(hf16_dst, so[:], v[:], ALU.mult)
                    if hf8_dst is not None:
                        nc.gpsimd.tensor_tensor(hf8_dst, so[:], v[:],
                                                ALU.mult)

              def emit_global_j(b, l, j):
                    tid = ("g", l, j)
                    if (b, l) not in h16:
                        h16[(b, l)] = hpool.tile([P, 4, SB], F16,
                                                 name=f"h16_{l}_{b}",
                                                 tag=f"h16_{l}", bufs=2)
                        if l < 2:
                            h8s[(b, l)] = hpool.tile([P, 4, SB], F8,
                                                     name=f"h8_{l}_{b}",
                                                     tag=f"h8_{l}", bufs=2)
                    hf = h16[(b, l)]
                    h8 = h8s.get((b, l))
                    zt = zts[b]
                    ps = {}
                    if l == 0:
                        for g in GATES:
                            psg = pspool.tile([P, SB], F32, tag="gate", bufs=3)
                            for nb in range(NBK):
                                nc.tensor.matmul(
                                    psg[:, nb * 512:(nb + 1) * 512],
                                    w16("g", 0, g, j, 0),
                                    zt[:, nb * 512:(nb + 1) * 512],
                                    start=True, stop=True)
                            ps[g] = psg
                    else:
                        hp16 = h16[(b, l - 1)]
                        hp8 = h8s[(b, l - 1)]
                        for g in ("i", "c", "o"):
                            psg = pspool.tile([P, SB], F32, tag="gate", bufs=3)
                            if g == "c":
                                for nb in range(NBK):
                                    for k in range(4):
                                        nc.tensor.matmul(
                                            psg[:, nb * 512:(nb + 1) * 512],
                                            w16("g", l, "c", j, k),
                                            hp16[:, k, nb * 512:(nb + 1) * 512],
                                            start=(k == 0), stop=(k == 3))
                            else:
                                for nb in range(NBK):
                                    for kp in range(2):
                                        nc.tensor.matmul(
                                            psg[:, nb * 512:(nb + 1) * 512],
                                            w8(l, g, j, kp),
                                            hp8[:, 2 * kp:2 * kp + 2,
                                                nb * 512:(nb + 1) * 512],
                                            start=(kp == 0), stop=(kp == 1),
                                            perf_mode=DR)
                            ps[g] = psg
                    emit_tail(tid, "g", l,
                              lambda g: bcol("g", l, g, j),
                              ps["i"], ps["c"], ps["o"],
                              hf[:, j, :],
                              h8[:, j, :] if h8 is not None else None,
                              l == 2)

              def emit_noise(b, l):
                    tid = ("n", l, 0)
                    hcur = hpool.tile([P, SB], F16, tag=f"hn{l}", bufs=2)
                    src = zts[b] if l == 0 else hns[(b, 0)]
                    ps = {}
                    for g in GATES:
                        psg = pspool.tile([P, SB], F32, tag="gate", bufs=3)
                        for nb in range(NBK):
                            nc.tensor.matmul(
                                psg[:, nb * 512:(nb + 1) * 512],
                                w16("n", l, g, 0, 0),
                                src[:, nb * 512:(nb + 1) * 512],
                                start=True, stop=True)
                        ps[g] = psg
                    emit_tail(tid, "n", l,
                              lambda g: bcol("n", l, g, 0),
                              ps["i"], ps["c"], ps["o"],
                              hcur[:], None, l == 1)
                    hns[(b, l)] = hcur

              def emit_st(b):
                    hnf = hns.pop((b, 1))
                    st_ps = pspool.tile([P, CHUNKS], F32, tag="sm")
                    for c in range(CHUNKS):
                        nc.tensor.matmul(st_ps[:, c:c + 1],
                                         hnf[:, c * P:(c + 1) * P],
                                         aw_col, start=True, stop=True)
                    nc.vector.tensor_copy(
                        stsb[:, b * CHUNKS:(b + 1) * CHUNKS], st_ps[:])

              def emit_mu(b):
                    gt = h16.pop((b, 2))
                    mu_ps = pspool.tile([P, CHUNKS], F32, tag="sm")
                    for c in range(CHUNKS):
                        for j in range(4):
                            nc.tensor.matmul(
                                mu_ps[:, c:c + 1],
                                gt[:, j, c * P:(c + 1) * P],
                                wsum_cols[j],
                                start=(j == 0), stop=(j == 3))
                    nc.vector.tensor_scalar_add(
                        musb[:, b * CHUNKS:(b + 1) * CHUNKS], mu_ps[:], b_sum)

              fetch_zt(0)
              for s in range(nsb + 2):
                b0, b1, b2 = s, s - 1, s - 2
                if s + 1 < nsb:
                    fetch_zt(s + 1)
                seq = []
                for j in range(4):
                    if 0 <= b2 < nsb:
                        seq.append(("g2", b2, j))
                    if 0 <= b1 < nsb:
                        seq.append(("g1", b1, j))
                    if b0 < nsb:
                        seq.append(("g0", b0, j))
                if b0 < nsb:
                    seq.append(("n0", b0, 0))
                if 0 <= b1 < nsb:
                    seq.append(("n1", b1, 0))
                for kind, b, j in seq:
                    if kind == "g0":
                        emit_global_j(b, 0, j)
                    elif kind == "g1":
                        emit_global_j(b, 1, j)
                    elif kind == "g2":
                        emit_global_j(b, 2, j)
                    elif kind == "n0":
                        emit_noise(b, 0)
                    elif kind == "n1":
                        emit_noise(b, 1)
                if 0 <= b2 < nsb:
                    emit_st(b2)
                    emit_mu(b2)
                    h16.pop((b2, 1), None)
                    h8s.pop((b2, 1), None)
                    hns.pop((b2, 0), None)
                    zts.pop(b2, None)
                    h16.pop((b2, 0), None)
                    h8s.pop((b2, 0), None)

              # ---- epilogue: sigma = ln(1 + exp(st + a_b)) + 1e-6 ----
              nc.scalar.activation(sgsb[:], stsb[:], AF.Exp, bias=a_bias)
              nc.scalar.activation(sgsb[:], sgsb[:], AF.Ln, bias=1.0)
              nc.vector.tensor_scalar_add(sgsb[:], sgsb[:], 1e-6)
              nc.sync.dma_start(mu_d, musb[:])
              nc.sync.dma_start(sg_d, sgsb[:])

    nc.compile()
    return nc


def _unshuffle(arr, tok):
    nsb = tok // SB
    return (arr.reshape(P, nsb, CHUNKS).transpose(1, 2, 0).reshape(tok))


def run(X, weights_kwargs, tok_per_core, n_cores, trace=False):
    wt16, wt8, bias, b_sum, a_bias = pack_host(**weights_kwargs)
    nc = build_nc(tok_per_core, b_sum, a_bias)
    xt = pack_x(X)
    in_maps = []
    for c in range(n_cores):
        shard = xt[:, c * tok_per_core:(c + 1) * tok_per_core]
        in_maps.append({"x": np.ascontiguousarray(shard),
                        "wt16": wt16, "wt8": wt8, "bias": bias})
    res = bass_utils.run_bass_kernel_spmd(
        nc, in_maps, core_ids=list(range(n_cores)), trace=trace)
    mus, sgs = [], []
    for c in range(n_cores):
        mus.append(_unshuffle(res.results[c]["mu"], tok_per_core))
        sgs.append(_unshuffle(res.results[c]["sigma"], tok_per_core))
    nt = X.shape[0]
    mu = np.concatenate(mus).reshape(nt, T).astype(np.float32)
    sg = np.concatenate(sgs).reshape(nt, T).astype(np.float32)
    return mu, sg, res


def kernel(X, g_Wih0, g_bih0, g_bhh0, g_Wih, g_bih, g_bhh, e_W, e_b,
           n_Wih0, n_bih0, n_bhh0, n_Wih, n_bih, n_bhh, a_W, a_b):
    wk = dict(g_Wih0=g_Wih0, g_bih0=g_bih0, g_bhh0=g_bhh0, g_Wih=g_Wih,
              g_bih=g_bih, g_bhh=g_bhh, e_W=e_W, e_b=e_b, n_Wih0=n_Wih0,
              n_bih0=n_bih0, n_bhh0=n_bhh0, n_Wih=n_Wih, n_bih=n_bih,
              n_bhh=n_bhh, a_W=a_W, a_b=a_b)
    tok = FULL_NT * T // NCORES      # 24576
    mu, sg, _ = run(X, wk, tok, NCORES)
    return mu, sg
